# revision 5
# baseline (speedup 1.0000x reference)
"""Cross-attention kernel for 8 TRN2 NeuronCores (Bass/Tile).

Reference computation (fp32):
    q = x @ Wq; k = ctx @ Wk; v = ctx @ Wv        (reshape to heads, d=64)
    sim = q k^T * d^-0.5 ; attn = softmax(sim)
    out = (attn v) @ Wo + bo

Shapes: x [4, 2048, 1024], context [4, 1024, 768], 8 heads * 64, out [4, 2048, 1024].

Sharding (data ||): core c -> batch b=c//2, head-group hg=c%2 (4 heads).
Each core computes a partial output [2048, 1024] = attn-out(4 heads) @ Wo[hg-slice];
host sums the two partials per batch and adds bo.  No FLOP duplication
(5.1 GFLOP/core = total/8).

Per-core layout strategy (everything f32r on the matmul path):
  - host passes x^T [1024f, 2048i] and ctx^T [768f, 1024j]  (contraction dims on
    partitions; avoids any on-device transposes)
  - QT[d,i] = matmul(lhsT=Wq, rhs=xT);  KT[d,j] = matmul(lhsT=Wk, rhs=ctxT)
  - V[j,d]  = matmul(lhsT=ctxT, rhs=Wv), stored per-head as [V_h | 1] (ones col)
  - simT[j,i] = matmul(lhsT=KT_h, rhs=QT_h)        (K=64 contraction)
  - expT = exp(0.125*simT)  (ScalarE, unsafe softmax: |sim|*scale < ~4, exact)
  - av' = matmul(lhsT=[V_h|1], rhs=expT) accumulated over j -> [65, i]:
        rows 0..63 = unnormalised attn-out^T, row 64 = softmax denominator
  - recip(denom) broadcast j->partitions via DRAM round-trip DMA;
    out2T = av'[0:64] * recip  (DVE)  -> exactly the lhsT layout Wo needs
  - out[i,e] = matmul(lhsT=out2T, rhs=Wo)  -> DMA to DRAM (f32 partial)
"""

import numpy as np

import concourse.bass as bass
import concourse.tile as tile
from concourse import bacc, mybir
from concourse.bass_utils import run_bass_kernel_spmd

# problem constants (hardcoded per the harness contract)
B = 4              # batches
I = 2048           # query seq len
J = 1024           # context seq len
FQ = 1024          # query dim
FC = 768           # context dim
DH = 64            # head dim
HPC = 4            # heads per core
DG = HPC * DH      # 256: per-core slice of inner dim
E = 1024           # output dim
P = 128
N_CORES = 8
IH = I // 2        # 1024: i-half processed per attention pass

F32 = mybir.dt.float32
F32R = mybir.dt.float32r

KQ = FQ // P       # 8  k-blocks for q projection
KC = FC // P       # 6  k-blocks for k/v projection
TD = DG // P       # 2  partition-blocks of the per-core inner dim
JBN = J // P       # 8  j-blocks


def _build():
    nc = bacc.Bacc()
    xt = nc.declare_dram_parameter("xt", [FQ, I], F32R, isOutput=False)
    ctxt = nc.declare_dram_parameter("ctxt", [FC, J], F32R, isOutput=False)
    wq = nc.declare_dram_parameter("wq", [FQ, DG], F32R, isOutput=False)
    wk = nc.declare_dram_parameter("wk", [FC, DG], F32R, isOutput=False)
    wv = nc.declare_dram_parameter("wv", [FC, DG], F32R, isOutput=False)
    wo = nc.declare_dram_parameter("wo", [DG, E], F32R, isOutput=False)
    vones = nc.declare_dram_parameter("vones", [P, HPC], F32R, isOutput=False)
    out = nc.declare_dram_parameter("out", [I, E], F32, isOutput=True)
    brc = nc.dram_tensor("brc", [2 * HPC, IH], F32)  # recip rows for bcast

    with tile.TileContext(nc) as tc:
        with (
            tc.tile_pool(name="consts", bufs=1) as consts,
            tc.tile_pool(name="xtp", bufs=2) as xtp,
            tc.tile_pool(name="expp", bufs=6) as expp,
            tc.tile_pool(name="misc", bufs=2) as misc,
            tc.tile_pool(name="outp", bufs=3) as outp,
            tc.tile_pool(name="pp", bufs=2, space="PSUM") as pp,
            tc.tile_pool(name="avp", bufs=1, space="PSUM") as avpool,
            tc.tile_pool(name="wop", bufs=2, space="PSUM") as wop,
        ):
            # ---------------- persistent loads ----------------
            ctxt_sb = consts.tile([P, KC, J], F32R, tag="ctxt_sb")
            nc.gpsimd.dma_start(
                out=ctxt_sb, in_=ctxt[:, :].rearrange("(kb p) j -> p kb j", p=P)
            )
            wq_sb = consts.tile([P, KQ, DG], F32R, tag="wq_sb")
            nc.gpsimd.dma_start(
                out=wq_sb, in_=wq[:, :].rearrange("(kb p) d -> p kb d", p=P)
            )
            wk_sb = consts.tile([P, KC, DG], F32R, tag="wk_sb")
            nc.gpsimd.dma_start(
                out=wk_sb, in_=wk[:, :].rearrange("(kb p) d -> p kb d", p=P)
            )
            wv_sb = consts.tile([P, KC, DG], F32R, tag="wv_sb")
            nc.gpsimd.dma_start(
                out=wv_sb, in_=wv[:, :].rearrange("(kb p) d -> p kb d", p=P)
            )
            wo_sb = consts.tile([P, TD, E], F32R, tag="wo_sb")
            nc.gpsimd.dma_start(
                out=wo_sb, in_=wo[:, :].rearrange("(kb p) e -> p kb e", p=P)
            )

            # ---------------- projections ----------------
            # KT [d=256, j=1024]: lhsT = wk [f, d], rhs = ctxT [f, j]
            kt_sb = [consts.tile([P, J], F32R, tag=f"kt{t}", name=f"kt{t}") for t in range(TD)]
            for t in range(TD):
                ps = pp.tile([P, IH], F32, tag="pp")
                for kb in range(KC):
                    for nchunk in range(2):
                        nc.tensor.matmul(
                            ps[:, nchunk * 512:(nchunk + 1) * 512],
                            lhsT=wk_sb[:, kb, t * P:(t + 1) * P],
                            rhs=ctxt_sb[:, kb, nchunk * 512:(nchunk + 1) * 512],
                            start=(kb == 0), stop=(kb == KC - 1),
                        )
                nc.vector.tensor_copy(kt_sb[t], ps)

            # V [j, d] per-head with ones column: v_sb[jb] = [128, HPC, 65]
            v_sb = [consts.tile([P, HPC, DH + 1], F32R, tag=f"v{jb}", name=f"v{jb}")
                    for jb in range(JBN)]
            for jb in range(JBN):
                nc.gpsimd.dma_start(
                    out=v_sb[jb][:, :, DH:DH + 1],
                    in_=vones[:, :].rearrange("p (h o) -> p h o", o=1),
                )
                ps = wop.tile([P, 512], F32, tag="wops")
                for kb in range(KC):
                    nc.tensor.matmul(
                        ps[:, :DG],
                        lhsT=ctxt_sb[:, kb, jb * P:(jb + 1) * P],
                        rhs=wv_sb[:, kb, :],
                        start=(kb == 0), stop=(kb == KC - 1),
                    )
                nc.vector.tensor_copy(
                    v_sb[jb][:, :, 0:DH],
                    ps[:, :DG].rearrange("p (h d) -> p h d", h=HPC),
                )

            # QT [d=256, i=2048]: lhsT = wq [f, d], rhs = xT [f, i]
            qt_sb = [consts.tile([P, I], F32R, tag=f"qt{t}", name=f"qt{t}") for t in range(TD)]
            for ich in range(4):
                isl = slice(ich * 512, (ich + 1) * 512)
                xtile = xtp.tile([P, KQ, 512], F32R, tag="xt")
                nc.gpsimd.dma_start(
                    out=xtile,
                    in_=xt[:, isl].rearrange("(kb p) i -> p kb i", p=P),
                )
                for t in range(TD):
                    ps = pp.tile([P, IH], F32, tag="pp")
                    for kb in range(KQ):
                        nc.tensor.matmul(
                            ps[:, :512],
                            lhsT=wq_sb[:, kb, t * P:(t + 1) * P],
                            rhs=xtile[:, kb, :],
                            start=(kb == 0), stop=(kb == KQ - 1),
                        )
                    nc.vector.tensor_copy(qt_sb[t][:, isl], ps[:, :512])

            # ---------------- attention + output projection ----------------
            o2t_sb = [[consts.tile([P, IH], F32R, tag=f"o2t{half}{t}", name=f"o2t{half}{t}")
                       for t in range(TD)] for half in range(2)]

            for half in range(2):
                for h in range(HPC):
                    t, prow = h // 2, (h % 2) * DH
                    av = avpool.tile([DH + 1, IH], F32, tag="av")
                    for jb in range(JBN):
                        sc = pp.tile([P, IH], F32, tag="pp")
                        for nchunk in range(2):
                            csl = slice(nchunk * 512, (nchunk + 1) * 512)
                            qsl = slice(half * IH + nchunk * 512,
                                        half * IH + (nchunk + 1) * 512)
                            nc.tensor.matmul(
                                sc[:, csl],
                                lhsT=kt_sb[t][prow:prow + DH, jb * P:(jb + 1) * P],
                                rhs=qt_sb[t][prow:prow + DH, qsl],
                                start=True, stop=True,
                            )
                        et = expp.tile([P, IH], F32R, tag="et")
                        nc.scalar.activation(
                            out=et, in_=sc,
                            func=mybir.ActivationFunctionType.Exp, scale=0.125,
                        )
                        for nchunk in range(2):
                            csl = slice(nchunk * 512, (nchunk + 1) * 512)
                            nc.tensor.matmul(
                                av[:, csl],
                                lhsT=v_sb[jb][:, h, :],
                                rhs=et[:, csl],
                                start=(jb == 0), stop=(jb == JBN - 1),
                            )
                    # normalise: out2T = av[0:64] / av[64]
                    araw = misc.tile([DH + 1, IH], F32, tag="araw")
                    nc.vector.tensor_copy(araw, av)
                    rr = misc.tile([1, IH], F32, tag="rr")
                    nc.vector.reciprocal(rr, araw[DH:DH + 1, :])
                    bidx = half * HPC + h
                    nc.sync.dma_start(out=brc[bidx:bidx + 1, :], in_=rr)
                    bc = misc.tile([DH, IH], F32, tag="bc")
                    row = brc[bidx:bidx + 1, :]
                    nc.sync.dma_start(
                        out=bc,
                        in_=bass.AP(tensor=row.tensor, offset=row.offset,
                                    ap=[[0, DH]] + row.ap[1:]),
                    )
                    nc.vector.tensor_mul(
                        o2t_sb[half][t][prow:prow + DH, :], araw[0:DH, :], bc
                    )

                # Wo projection for this half: out[i, e] = out2T.T @ Wo
                for m in range(IH // P):
                    ot = outp.tile([P, E], F32, tag="ot")
                    pss = [wop.tile([P, 512], F32, tag="wops", name=f"wops{m}") for _ in range(2)]
                    for t in range(TD):
                        for nchunk in range(2):
                            nc.tensor.matmul(
                                pss[nchunk],
                                lhsT=o2t_sb[half][t][:, m * P:(m + 1) * P],
                                rhs=wo_sb[:, t, nchunk * 512:(nchunk + 1) * 512],
                                start=(t == 0), stop=(t == TD - 1),
                            )
                    for nchunk in range(2):
                        nc.vector.tensor_copy(
                            ot[:, nchunk * 512:(nchunk + 1) * 512], pss[nchunk]
                        )
                    r0 = half * IH + m * P
                    nc.sync.dma_start(out=out[r0:r0 + P, :], in_=ot)

    nc.compile()
    return nc


_NC_CACHE = None


def _get_nc():
    global _NC_CACHE
    if _NC_CACHE is None:
        _NC_CACHE = _build()
    return _NC_CACHE


def _make_in_maps(x, context, Wq, Wk, Wv, Wo):
    in_maps = []
    for c in range(N_CORES):
        b, hg = c // 2, c % 2
        sl = slice(hg * DG, (hg + 1) * DG)
        in_maps.append({
            "xt": np.ascontiguousarray(x[b].T),
            "ctxt": np.ascontiguousarray(context[b].T),
            "wq": np.ascontiguousarray(Wq[:, sl]),
            "wk": np.ascontiguousarray(Wk[:, sl]),
            "wv": np.ascontiguousarray(Wv[:, sl]),
            "wo": np.ascontiguousarray(Wo[sl, :]),
            "vones": np.ones((P, HPC), dtype=np.float32),
        })
    return in_maps


def _run(inputs, trace=False):
    x = np.asarray(inputs["x"], dtype=np.float32)
    context = np.asarray(inputs["context"], dtype=np.float32)
    Wq = np.asarray(inputs["Wq"], dtype=np.float32)
    Wk = np.asarray(inputs["Wk"], dtype=np.float32)
    Wv = np.asarray(inputs["Wv"], dtype=np.float32)
    Wo = np.asarray(inputs["Wo"], dtype=np.float32)
    bo = np.asarray(inputs["bo"], dtype=np.float32)

    res = run_bass_kernel_spmd(
        _get_nc(), _make_in_maps(x, context, Wq, Wk, Wv, Wo),
        core_ids=list(range(N_CORES)), trace=trace,
    )
    parts = [np.asarray(r["out"], dtype=np.float32) for r in res.results]
    outv = np.stack([parts[2 * b] + parts[2 * b + 1] + bo for b in range(B)])
    return outv, res


def kernel(**inputs) -> np.ndarray:
    outv, _ = _run(inputs, trace=False)
    return outv


# revision 6
# speedup vs baseline: 1.5995x; 1.5995x over previous
"""Cross-attention kernel for 8 TRN2 NeuronCores (Bass/Tile).

Reference computation (fp32):
    q = x @ Wq; k = ctx @ Wk; v = ctx @ Wv        (reshape to heads, d=64)
    sim = q k^T * d^-0.5 ; attn = softmax(sim)
    out = (attn v) @ Wo + bo

Shapes: x [4, 2048, 1024], context [4, 1024, 768], 8 heads * 64, out [4, 2048, 1024].

Sharding (data ||): core c -> batch b=c//2, head-group hg=c%2 (4 heads).
Each core computes a partial output [2048, 1024] = attn-out(4 heads) @ Wo[hg-slice];
host sums the two partials per batch and adds bo.  No FLOP duplication
(5.1 GFLOP/core = total/8).

Per-core layout strategy (bf16 matmul operands, fp32 accumulation):
  - host passes x^T [1024f, 2048i] and ctx^T [768f, 1024j]  (contraction dims on
    partitions; avoids any on-device transposes)
  - QT[d,i] = matmul(lhsT=Wq, rhs=xT);  KT[d,j] = matmul(lhsT=Wk, rhs=ctxT)
  - V[j,d]  = matmul(lhsT=ctxT, rhs=Wv), stored per-head as [V_h | 1] (ones col)
  - simT[j,i] = matmul(lhsT=KT_h, rhs=QT_h)        (K=64 contraction)
  - expT = exp(0.125*simT)  (ScalarE, unsafe softmax: |sim|*scale < ~4, exact)
  - av' = matmul(lhsT=[V_h|1], rhs=expT) accumulated over j -> [65, i]:
        rows 0..63 = unnormalised attn-out^T, row 64 = softmax denominator
  - recip(denom) broadcast j->partitions via DRAM round-trip DMA;
    out2T = av'[0:64] * recip  (DVE)  -> exactly the lhsT layout Wo needs
  - out[i,e] = matmul(lhsT=out2T, rhs=Wo)  -> DMA to DRAM (f32 partial)
"""

import numpy as np
import ml_dtypes

import concourse.bass as bass
import concourse.tile as tile
from concourse import bacc, mybir
from concourse.bass_utils import run_bass_kernel_spmd

# problem constants (hardcoded per the harness contract)
B = 4              # batches
I = 2048           # query seq len
J = 1024           # context seq len
FQ = 1024          # query dim
FC = 768           # context dim
DH = 64            # head dim
HPC = 4            # heads per core
DG = HPC * DH      # 256: per-core slice of inner dim
E = 1024           # output dim
P = 128
N_CORES = 8
IH = I // 2        # 1024: i-half processed per attention pass

F32 = mybir.dt.float32
BF16 = mybir.dt.bfloat16

KQ = FQ // P       # 8  k-blocks for q projection
KC = FC // P       # 6  k-blocks for k/v projection
TD = DG // P       # 2  partition-blocks of the per-core inner dim
JBN = J // P       # 8  j-blocks


def _build():
    nc = bacc.Bacc()
    xt = nc.declare_dram_parameter("xt", [FQ, I], BF16, isOutput=False)
    ctxt = nc.declare_dram_parameter("ctxt", [FC, J], BF16, isOutput=False)
    wq = nc.declare_dram_parameter("wq", [FQ, DG], BF16, isOutput=False)
    wk = nc.declare_dram_parameter("wk", [FC, DG], BF16, isOutput=False)
    wv = nc.declare_dram_parameter("wv", [FC, DG], BF16, isOutput=False)
    wo = nc.declare_dram_parameter("wo", [DG, E], BF16, isOutput=False)
    vones = nc.declare_dram_parameter("vones", [P, HPC], BF16, isOutput=False)
    out = nc.declare_dram_parameter("out", [I, E], F32, isOutput=True)
    brc = nc.dram_tensor("brc", [2 * HPC, IH], F32)   # denom rows (j->dram)
    brc2 = nc.dram_tensor("brc2", [2 * HPC, IH], F32)  # recip rows for bcast

    with tile.TileContext(nc) as tc:
        with (
            tc.tile_pool(name="consts", bufs=1) as consts,
            tc.tile_pool(name="xtp", bufs=2) as xtp,
            tc.tile_pool(name="expp", bufs=6) as expp,
            tc.tile_pool(name="misc", bufs=2) as misc,
            tc.tile_pool(name="outp", bufs=3) as outp,
            tc.tile_pool(name="pp", bufs=2, space="PSUM") as pp,
            tc.tile_pool(name="avp", bufs=1, space="PSUM") as avpool,
            tc.tile_pool(name="wop", bufs=2, space="PSUM") as wop,
        ):
            # ---------------- persistent loads ----------------
            ctxt_sb = consts.tile([P, KC, J], BF16, tag="ctxt_sb")
            nc.gpsimd.dma_start(
                out=ctxt_sb, in_=ctxt[:, :].rearrange("(kb p) j -> p kb j", p=P)
            )
            wq_sb = consts.tile([P, KQ, DG], BF16, tag="wq_sb")
            nc.gpsimd.dma_start(
                out=wq_sb, in_=wq[:, :].rearrange("(kb p) d -> p kb d", p=P)
            )
            wk_sb = consts.tile([P, KC, DG], BF16, tag="wk_sb")
            nc.gpsimd.dma_start(
                out=wk_sb, in_=wk[:, :].rearrange("(kb p) d -> p kb d", p=P)
            )
            wv_sb = consts.tile([P, KC, DG], BF16, tag="wv_sb")
            nc.gpsimd.dma_start(
                out=wv_sb, in_=wv[:, :].rearrange("(kb p) d -> p kb d", p=P)
            )
            wo_sb = consts.tile([P, TD, E], BF16, tag="wo_sb")
            nc.gpsimd.dma_start(
                out=wo_sb, in_=wo[:, :].rearrange("(kb p) e -> p kb e", p=P)
            )

            # ---------------- projections ----------------
            # KT [d=256, j=1024]: lhsT = wk [f, d], rhs = ctxT [f, j]
            kt_sb = [consts.tile([P, J], BF16, tag=f"kt{t}", name=f"kt{t}") for t in range(TD)]
            for t in range(TD):
                ps = pp.tile([P, IH], F32, tag="pp")
                for kb in range(KC):
                    for nchunk in range(2):
                        nc.tensor.matmul(
                            ps[:, nchunk * 512:(nchunk + 1) * 512],
                            lhsT=wk_sb[:, kb, t * P:(t + 1) * P],
                            rhs=ctxt_sb[:, kb, nchunk * 512:(nchunk + 1) * 512],
                            start=(kb == 0), stop=(kb == KC - 1),
                        )
                nc.vector.tensor_copy(kt_sb[t], ps)

            # V [j, d] per-head with ones column: v_sb[jb] = [128, HPC, 65]
            v_sb = [consts.tile([P, HPC, DH + 1], BF16, tag=f"v{jb}", name=f"v{jb}")
                    for jb in range(JBN)]
            for jb in range(JBN):
                nc.gpsimd.dma_start(
                    out=v_sb[jb][:, :, DH:DH + 1],
                    in_=vones[:, :].rearrange("p (h o) -> p h o", o=1),
                )
                ps = wop.tile([P, 512], F32, tag="wops")
                for kb in range(KC):
                    nc.tensor.matmul(
                        ps[:, :DG],
                        lhsT=ctxt_sb[:, kb, jb * P:(jb + 1) * P],
                        rhs=wv_sb[:, kb, :],
                        start=(kb == 0), stop=(kb == KC - 1),
                    )
                nc.vector.tensor_copy(
                    v_sb[jb][:, :, 0:DH],
                    ps[:, :DG].rearrange("p (h d) -> p h d", h=HPC),
                )

            # QT [d=256, i=2048]: lhsT = wq [f, d], rhs = xT [f, i]
            qt_sb = [consts.tile([P, I], BF16, tag=f"qt{t}", name=f"qt{t}") for t in range(TD)]
            for ich in range(4):
                isl = slice(ich * 512, (ich + 1) * 512)
                xtile = xtp.tile([P, KQ, 512], BF16, tag="xt")
                nc.gpsimd.dma_start(
                    out=xtile,
                    in_=xt[:, isl].rearrange("(kb p) i -> p kb i", p=P),
                )
                for t in range(TD):
                    ps = pp.tile([P, IH], F32, tag="pp")
                    for kb in range(KQ):
                        nc.tensor.matmul(
                            ps[:, :512],
                            lhsT=wq_sb[:, kb, t * P:(t + 1) * P],
                            rhs=xtile[:, kb, :],
                            start=(kb == 0), stop=(kb == KQ - 1),
                        )
                    nc.vector.tensor_copy(qt_sb[t][:, isl], ps[:, :512])

            # ---------------- attention + output projection ----------------
            o2t_sb = [[consts.tile([P, IH], BF16, tag=f"o2t{half}{t}", name=f"o2t{half}{t}")
                       for t in range(TD)] for half in range(2)]

            for half in range(2):
                for h in range(HPC):
                    t, prow = h // 2, (h % 2) * DH
                    av = avpool.tile([DH + 1, IH], F32, tag="av")
                    for jb in range(JBN):
                        sc = pp.tile([P, IH], F32, tag="pp")
                        for nchunk in range(2):
                            csl = slice(nchunk * 512, (nchunk + 1) * 512)
                            qsl = slice(half * IH + nchunk * 512,
                                        half * IH + (nchunk + 1) * 512)
                            nc.tensor.matmul(
                                sc[:, csl],
                                lhsT=kt_sb[t][prow:prow + DH, jb * P:(jb + 1) * P],
                                rhs=qt_sb[t][prow:prow + DH, qsl],
                                start=True, stop=True,
                            )
                        et = expp.tile([P, IH], BF16, tag="et")
                        nc.scalar.activation(
                            out=et, in_=sc,
                            func=mybir.ActivationFunctionType.Exp, scale=0.125,
                        )
                        for nchunk in range(2):
                            csl = slice(nchunk * 512, (nchunk + 1) * 512)
                            nc.tensor.matmul(
                                av[:, csl],
                                lhsT=v_sb[jb][:, h, :],
                                rhs=et[:, csl],
                                start=(jb == 0), stop=(jb == JBN - 1),
                            )
                    # normalise: out2T = av[0:64] / av[64]
                    araw = misc.tile([DH + 1, IH], F32, tag="araw")
                    nc.vector.tensor_copy(araw, av)
                    bidx = half * HPC + h
                    # reciprocal is slow per-lane-element: transpose the denom
                    # row to [128, IH/128] via DRAM so all lanes work
                    nc.sync.dma_start(out=brc[bidx:bidx + 1, :],
                                      in_=araw[DH:DH + 1, :])
                    rcol = misc.tile([P, IH // P], F32, tag="rcol")
                    nc.sync.dma_start(
                        out=rcol,
                        in_=brc[bidx, :].rearrange("(p t) -> p t", p=P),
                    )
                    rrec = misc.tile([P, IH // P], F32, tag="rrec")
                    nc.vector.reciprocal(rrec, rcol)
                    nc.sync.dma_start(
                        out=brc2[bidx, :].rearrange("(p t) -> p t", p=P),
                        in_=rrec,
                    )
                    bc = misc.tile([DH, IH], F32, tag="bc")
                    row = brc2[bidx:bidx + 1, :]
                    nc.sync.dma_start(
                        out=bc,
                        in_=bass.AP(tensor=row.tensor, offset=row.offset,
                                    ap=[[0, DH]] + row.ap[1:]),
                    )
                    nc.vector.tensor_mul(
                        o2t_sb[half][t][prow:prow + DH, :], araw[0:DH, :], bc
                    )

                # Wo projection for this half: out[i, e] = out2T.T @ Wo
                for m in range(IH // P):
                    ot = outp.tile([P, E], F32, tag="ot")
                    pss = [wop.tile([P, 512], F32, tag="wops", name=f"wops{m}") for _ in range(2)]
                    for t in range(TD):
                        for nchunk in range(2):
                            nc.tensor.matmul(
                                pss[nchunk],
                                lhsT=o2t_sb[half][t][:, m * P:(m + 1) * P],
                                rhs=wo_sb[:, t, nchunk * 512:(nchunk + 1) * 512],
                                start=(t == 0), stop=(t == TD - 1),
                            )
                    for nchunk in range(2):
                        nc.vector.tensor_copy(
                            ot[:, nchunk * 512:(nchunk + 1) * 512], pss[nchunk]
                        )
                    r0 = half * IH + m * P
                    nc.sync.dma_start(out=out[r0:r0 + P, :], in_=ot)

    nc.compile()
    return nc


_NC_CACHE = None


def _get_nc():
    global _NC_CACHE
    if _NC_CACHE is None:
        _NC_CACHE = _build()
    return _NC_CACHE


def _make_in_maps(x, context, Wq, Wk, Wv, Wo):
    in_maps = []
    for c in range(N_CORES):
        b, hg = c // 2, c % 2
        sl = slice(hg * DG, (hg + 1) * DG)
        bf = ml_dtypes.bfloat16
        in_maps.append({
            "xt": np.ascontiguousarray(x[b].T).astype(bf),
            "ctxt": np.ascontiguousarray(context[b].T).astype(bf),
            "wq": np.ascontiguousarray(Wq[:, sl]).astype(bf),
            "wk": np.ascontiguousarray(Wk[:, sl]).astype(bf),
            "wv": np.ascontiguousarray(Wv[:, sl]).astype(bf),
            "wo": np.ascontiguousarray(Wo[sl, :]).astype(bf),
            "vones": np.ones((P, HPC), dtype=bf),
        })
    return in_maps


def _run(inputs, trace=False):
    x = np.asarray(inputs["x"], dtype=np.float32)
    context = np.asarray(inputs["context"], dtype=np.float32)
    Wq = np.asarray(inputs["Wq"], dtype=np.float32)
    Wk = np.asarray(inputs["Wk"], dtype=np.float32)
    Wv = np.asarray(inputs["Wv"], dtype=np.float32)
    Wo = np.asarray(inputs["Wo"], dtype=np.float32)
    bo = np.asarray(inputs["bo"], dtype=np.float32)

    res = run_bass_kernel_spmd(
        _get_nc(), _make_in_maps(x, context, Wq, Wk, Wv, Wo),
        core_ids=list(range(N_CORES)), trace=trace,
    )
    parts = [np.asarray(r["out"], dtype=np.float32) for r in res.results]
    outv = np.stack([parts[2 * b] + parts[2 * b + 1] + bo for b in range(B)])
    return outv, res


def kernel(**inputs) -> np.ndarray:
    outv, _ = _run(inputs, trace=False)
    return outv
